# revision 24
# baseline (speedup 1.0000x reference)
"""Trainium2 Bass kernel for nn_BasicBlock (3-layer GCN block with residual).

Math (per batch item, per conv):
    out = A @ (x @ W) + bias,  A = normalized adjacency (with self loops)
where A[c, r] = sum over edges r->c of dinv[r]*dinv[c] (dense N x N, shared
across batch, precomputed on host from the edge lists).

Block:
    a1 = relu(A_sp @ (x  @ W1) + b1)
    a2 = relu(A_tm @ (a1 @ W2) + b2)
    o3 =      A_sp @ (a2 @ W3) + b3
    out = relu(o3 + x)

All matmuls run in fp8e4m3 with MatmulPerfMode.DoubleRow: operands are laid
out as [128, kt2, 2, F] with contraction row m = k2*256 + i*128 + p; each
matmul consumes 2 k-tiles (256 contraction rows) per pass, halving the pass
count vs bf16. The DR k-tile dim stride must be a multiple of 16 elements,
so node-extent tensors are padded to nl = roundup(n, 16). The residual path
stays bf16 (exact), so fp8 noise only enters through the conv stack;
measured rel err ~2.8e-3 vs the 2e-2 gate.

On-chip layouts per item (P=128 partitions):
    channel-major [cb, n]: channels on partitions (A-matmul transposed form,
                           W-matmul rhs/lhsT)
    natural    [n-chunk, c]: nodes on partitions (final conv + residual)

Phases per item:
    1. g1T[c,n]  = (A_sp @ x)^T        DR: lhsT=x8 chunk,   rhs=AT8_sp
    2. a1T[co,n] = relu(W1^T g1T + b1) DR: lhsT=W1,         rhs=g1T (1 pass)
    3. h2[n,c]   = a1 @ W2             DR: lhsT=a1T chunk,  rhs=W2   (1 pass)
    4. a2T[c,n]  = relu((A_tm h2)^T + b2)
    5. h3[n,c]   = a2 @ W3; h3[node N] = b3
    6. out[n,c]  = relu(A_sp @ h3 + x)   (AT_sp row N is all-ones -> + b3;
       items are processed in pairs sharing the stationary AT_sp operand,
       with the output stored pair-interleaved [pair, n, 2, c])

DMA rings: sync+scalar HWDGE carry the adjacency matrices and the output
stores; all x traffic (fp8 and bf16 copies) rides the gpsimd SWDGE queue.
Batch (64) is sharded 8 items/core over 8 cores; A/W/b replicated.
"""

import sys

if "/opt/trn_rl_repo" not in sys.path:
    sys.path.insert(0, "/opt/trn_rl_repo")

import numpy as np
import ml_dtypes

import concourse.bass as bass
import concourse.bacc as bacc
import concourse.mybir as mybir
import concourse.tile as tile
from concourse.bass_utils import run_bass_kernel_spmd

P = 128
B, N, C = 64, 1700, 256
N_CORES = 8
B_LOCAL = B // N_CORES

F32 = mybir.dt.float32
BF16 = mybir.dt.bfloat16
F8 = mybir.dt.float8e4
RELU = mybir.ActivationFunctionType.Relu
DR = mybir.MatmulPerfMode.DoubleRow
NP_BF16 = ml_dtypes.bfloat16
NP_F8 = ml_dtypes.float8_e4m3


def _quarters(total, parts=4):
    step = -(-total // parts)
    return [(q, min(step, total - q)) for q in range(0, total, step)]


def build_program(bl, n, c):
    """Build the Bass/Tile program for `bl` batch items, `n` nodes, `c` chans."""
    kt = -(-(n + 1) // P)          # 128-row node chunks incl bias row
    if kt % 2:
        kt += 1                    # DoubleRow needs an even chunk count
    npad = kt * P
    kt2 = kt // 2                  # 256-row DoubleRow passes
    ct = c // P
    nl = -(-n // 16) * 16          # node extent padded so DR k-tile strides %16
    nq = _quarters(n)

    # bias-row (node n) coordinates in the [P, kt2, 2, *] layout
    bias_k2, bias_r = divmod(n, 2 * P)
    bias_i, bias_p = divmod(bias_r, P)

    paired = bl % 2 == 0

    nc = bacc.Bacc("TRN2", target_bir_lowering=False, debug=False,
                   enable_asserts=False)

    x8_d = nc.dram_tensor("x8", [bl, P, kt2, 2, c], F8, kind="ExternalInput")
    xbf_d = nc.dram_tensor("xbf", [bl, n, c], BF16, kind="ExternalInput")
    atsp_d = nc.dram_tensor("at_sp", [P, kt2, 2, nl], F8, kind="ExternalInput")
    attm_d = nc.dram_tensor("at_tm", [P, kt2, 2, nl], F8, kind="ExternalInput")
    w_d = [nc.dram_tensor(f"w{i}", [P, 2, c], F8, kind="ExternalInput")
           for i in (1, 2, 3)]
    b1_d = nc.dram_tensor("b1", [P, ct], F32, kind="ExternalInput")
    b2_d = nc.dram_tensor("b2", [P, ct], F32, kind="ExternalInput")
    b3_d = nc.dram_tensor("b3", [1, c], F8, kind="ExternalInput")
    if paired:
        out_d = nc.dram_tensor("out", [bl // 2, n, 2, c], BF16,
                               kind="ExternalOutput")
    else:
        out_d = nc.dram_tensor("out", [bl, n, c], BF16, kind="ExternalOutput")

    with tile.TileContext(nc) as tc:
        with (
            tc.tile_pool(name="const", bufs=1) as cpool,
            tc.tile_pool(name="x8p", bufs=4) as x8p,
            tc.tile_pool(name="xbf", bufs=2) as xbfp,
            tc.tile_pool(name="act", bufs=8) as actp,
            tc.tile_pool(name="h", bufs=2) as hp,
            tc.tile_pool(name="hpair", bufs=2) as hpp,
            tc.tile_pool(name="outp", bufs=4) as outp,
            tc.tile_pool(name="psA", bufs=4, space="PSUM") as psA,
            tc.tile_pool(name="psW", bufs=4, space="PSUM") as psW,
        ):
            # --- constants. at_sp is consumed first (item-0 phase 1):
            # split each k2-chunk across sync+scalar HWDGE rings AND the
            # gpsimd SWDGE queue so chunk k2 lands roughly when the PE
            # reaches it. Small constants (w, b) go first on scalar. ---
            at_sp = cpool.tile([P, kt2, 2, nl], F8, tag="at_sp")
            at_tm = cpool.tile([P, kt2, 2, nl], F8, tag="at_tm")
            nh = nl // 2
            nt = (nl // 3) // 16 * 16

            w_sb = []
            for i, wd in enumerate(w_d):
                w = cpool.tile([P, 2, c], F8, tag=f"w{i}")
                nc.scalar.dma_start(w[:], wd[:])
                w_sb.append(w)
            b1_sb = cpool.tile([P, ct], F32, tag="b1")
            b2_sb = cpool.tile([P, ct], F32, tag="b2")
            nc.scalar.dma_start(b1_sb[:], b1_d[:])
            nc.scalar.dma_start(b2_sb[:], b2_d[:])

            x8_first = x8p.tile([P, kt2, 2, c], F8, tag="x8", name="x8_0")
            nc.sync.dma_start(x8_first[:], x8_d[0])

            for k2 in range(kt2):
                nc.sync.dma_start(at_sp[:, k2, :, :nt], atsp_d[:, k2, :, :nt])
                nc.scalar.dma_start(at_sp[:, k2, :, nt:2 * nt],
                                    atsp_d[:, k2, :, nt:2 * nt])
                nc.gpsimd.dma_start(at_sp[:, k2, :, 2 * nt:],
                                    atsp_d[:, k2, :, 2 * nt:])

            def emit_load_at_tm():
                # queued behind at_sp on both rings; first use is item-0 P4
                for k2 in range(kt2):
                    nc.sync.dma_start(at_tm[:, k2, :, :nh],
                                      attm_d[:, k2, :, :nh])
                    nc.scalar.dma_start(at_tm[:, k2, :, nh:],
                                        attm_d[:, k2, :, nh:])

            def emit_load_x8(b, eng=None):
                x8 = x8p.tile([P, kt2, 2, c], F8, tag="x8", name=f"x8_{b}")
                (eng or nc.gpsimd).dma_start(x8[:], x8_d[b])
                return x8

            def emit_load_xpair(b0, np_items):
                # residual x for a pair, interleaved [P, kt, np*c]
                xp = xbfp.tile([P, kt, np_items * c], BF16, tag="xbf",
                               name=f"xp_{b0}")
                for ip in range(np_items):
                    for k in range(kt):
                        rows = min(P, n - k * P)
                        if rows <= 0:
                            continue
                        nc.gpsimd.dma_start(
                            xp[:rows, k, ip * c:(ip + 1) * c],
                            xbf_d[b0 + ip, k * P:k * P + rows, :])
                return xp

            def emit_p1(b, x8, head=False):
                # phase 1: g1T = (A_sp @ x)^T, channel-major fp8.
                # head=True: k2-outer over 8 parallel PSUM banks so chunk k2
                # of at_sp is consumed as soon as its DMA lands.
                g1T = actp.tile([P, ct, nl], F8, tag="act", name=f"g1T_{b}")
                if head:
                    groups = []
                    for cb in range(ct):
                        for gi, (q0, qs) in enumerate(nq):
                            pool, tg = ((psA, "psA")
                                        if (cb * len(nq) + gi) % 2 == 0
                                        else (psW, "psW"))
                            groups.append(
                                (pool.tile([P, 512], F32, tag=tg,
                                           name=f"ps1_{cb}_{gi}"), cb, q0, qs))
                    for k2 in range(kt2):
                        for (ps, cb, q0, qs) in groups:
                            nc.tensor.matmul(
                                ps[:, :qs],
                                lhsT=x8[:, k2, :, cb * P:(cb + 1) * P],
                                rhs=at_sp[:, k2, :, q0:q0 + qs],
                                start=(k2 == 0), stop=(k2 == kt2 - 1),
                                perf_mode=DR)
                    for (ps, cb, q0, qs) in groups:
                        nc.vector.tensor_copy(g1T[:, cb, q0:q0 + qs],
                                              ps[:, :qs])
                    return g1T
                for cb in range(ct):
                    for qp in range(0, len(nq), 2):
                        grp = [(psA.tile([P, 512], F32, tag="psA",
                                         name=f"ps1_{cb}_{qp}_{qi}"), q0, qs)
                               for qi, (q0, qs) in enumerate(nq[qp:qp + 2])]
                        for k2 in range(kt2):
                            for (ps, q0, qs) in grp:
                                nc.tensor.matmul(
                                    ps[:, :qs],
                                    lhsT=x8[:, k2, :, cb * P:(cb + 1) * P],
                                    rhs=at_sp[:, k2, :, q0:q0 + qs],
                                    start=(k2 == 0), stop=(k2 == kt2 - 1),
                                    perf_mode=DR)
                        for (ps, q0, qs) in grp:
                            nc.vector.tensor_copy(g1T[:, cb, q0:q0 + qs],
                                                  ps[:, :qs])
                return g1T

            def emit_p2(b, g1T):
                # phase 2: a1T = relu(W1^T @ g1T + b1), single DR pass
                a1T = actp.tile([P, ct, npad], F8, tag="act", name=f"a1T_{b}")
                nc.gpsimd.memset(a1T[:, :, n:npad], 0)
                for co in range(ct):
                    for (q0, qs) in nq:
                        ps = psW.tile([P, 512], F32, tag="psW")
                        nc.tensor.matmul(
                            ps[:, :qs],
                            lhsT=w_sb[0][:, :, co * P:(co + 1) * P],
                            rhs=g1T[:, :, q0:q0 + qs],
                            start=True, stop=True, perf_mode=DR)
                        nc.scalar.activation(a1T[:, co, q0:q0 + qs],
                                             ps[:, :qs], RELU,
                                             bias=b1_sb[:, co:co + 1])
                return a1T

            def emit_p3(b, a1T):
                # phase 3: h2 = a1 @ W2 into DR layout, single pass per
                # chunk; two chunks share one PSUM bank and one evac
                h2 = hp.tile([P, kt2, 2, c], F8, tag="h", name=f"h2_{b}")
                for k2 in range(kt2):
                    ps = psW.tile([P, 2 * c], F32, tag="psW")
                    for j in range(2):
                        nc.tensor.matmul(
                            ps[:, j * c:(j + 1) * c],
                            lhsT=a1T[:, :, (2 * k2 + j) * P:(2 * k2 + j + 1) * P],
                            rhs=w_sb[1][:, :, :],
                            start=True, stop=True, perf_mode=DR)
                    nc.vector.tensor_copy(h2[:, k2, :, :], ps[:, :])
                return h2

            def emit_p4(b, h2, head=False):
                # phase 4: a2T = relu((A_tm @ h2)^T + b2)
                a2T = actp.tile([P, ct, npad], F8, tag="act", name=f"a2T_{b}")
                nc.gpsimd.memset(a2T[:, :, n:npad], 0)
                if head:
                    groups = []
                    for cb in range(ct):
                        for gi, (q0, qs) in enumerate(nq):
                            pool, tg = ((psA, "psA")
                                        if (cb * len(nq) + gi) % 2 == 0
                                        else (psW, "psW"))
                            groups.append(
                                (pool.tile([P, 512], F32, tag=tg,
                                           name=f"ps4_{cb}_{gi}"), cb, q0, qs))
                    for k2 in range(kt2):
                        for (ps, cb, q0, qs) in groups:
                            nc.tensor.matmul(
                                ps[:, :qs],
                                lhsT=h2[:, k2, :, cb * P:(cb + 1) * P],
                                rhs=at_tm[:, k2, :, q0:q0 + qs],
                                start=(k2 == 0), stop=(k2 == kt2 - 1),
                                perf_mode=DR)
                    for (ps, cb, q0, qs) in groups:
                        nc.scalar.activation(a2T[:, cb, q0:q0 + qs],
                                             ps[:, :qs], RELU,
                                             bias=b2_sb[:, cb:cb + 1])
                    return a2T
                for cb in range(ct):
                    for qp in range(0, len(nq), 2):
                        grp = [(psA.tile([P, 512], F32, tag="psA",
                                         name=f"ps4_{cb}_{qp}_{qi}"), q0, qs)
                               for qi, (q0, qs) in enumerate(nq[qp:qp + 2])]
                        for k2 in range(kt2):
                            for (ps, q0, qs) in grp:
                                nc.tensor.matmul(
                                    ps[:, :qs],
                                    lhsT=h2[:, k2, :, cb * P:(cb + 1) * P],
                                    rhs=at_tm[:, k2, :, q0:q0 + qs],
                                    start=(k2 == 0), stop=(k2 == kt2 - 1),
                                    perf_mode=DR)
                        for (ps, q0, qs) in grp:
                            nc.scalar.activation(a2T[:, cb, q0:q0 + qs],
                                                 ps[:, :qs], RELU,
                                                 bias=b2_sb[:, cb:cb + 1])
                return a2T

            def emit_p5(b, a2T, h3p, ip):
                # phase 5: h3 = a2 @ W3 into flat slot ip of a 2-item pair;
                # two chunks share one PSUM bank and one evac
                for k2 in range(kt2):
                    ps = psW.tile([P, 2 * c], F32, tag="psW")
                    for j in range(2):
                        nc.tensor.matmul(
                            ps[:, j * c:(j + 1) * c],
                            lhsT=a2T[:, :, (2 * k2 + j) * P:(2 * k2 + j + 1) * P],
                            rhs=w_sb[2][:, :, :],
                            start=True, stop=True, perf_mode=DR)
                    nc.vector.tensor_copy(
                        h3p[:, k2, :, ip * c:(ip + 1) * c], ps[:, :])
                nc.sync.dma_start(
                    h3p[bias_p:bias_p + 1, bias_k2, bias_i,
                        ip * c:(ip + 1) * c], b3_d[:, :])

            def emit_p6(b0, xp, h3p, np_items):
                # phase 6: out = relu(A_sp @ h3 + x) over an item pair,
                # sharing the stationary AT_sp operand (F = np_items*c).
                # Output is stored pair-interleaved: one evac chain and one
                # store per node chunk.
                fw = np_items * c
                for kop in range(0, kt, 2):
                    grp = []
                    for ko in (kop, kop + 1):
                        if ko >= kt:
                            continue
                        rows = min(P, n - ko * P)
                        if rows <= 0:
                            continue
                        grp.append((psW.tile([P, fw], F32, tag="psW",
                                              name=f"ps6_{ko}"),
                                    ko, rows))
                    for k2 in range(kt2):
                        for (ps, ko, rows) in grp:
                            nc.tensor.matmul(
                                ps[:rows, :],
                                lhsT=at_sp[:, k2, :, ko * P:ko * P + rows],
                                rhs=h3p[:, k2, :, :fw],
                                start=(k2 == 0), stop=(k2 == kt2 - 1),
                                perf_mode=DR)
                    for (ps, ko, rows) in grp:
                        ot = outp.tile([P, fw], BF16, tag="o")
                        nc.vector.tensor_add(ot[:rows, :], ps[:rows, :],
                                             xp[:rows, ko, :])
                        nc.scalar.activation(ot[:rows, :], ot[:rows, :], RELU)
                        st_eng = nc.scalar if ko % 2 == 0 else nc.sync
                        if paired:
                            st_eng.dma_start(
                                out_d[b0 // 2, ko * P:ko * P + rows, :, :],
                                ot[:rows, :])
                        else:
                            st_eng.dma_start(
                                out_d[b0, ko * P:ko * P + rows, :],
                                ot[:rows, :])

            def emit_mid(b, g1T, h3p, ip):
                a1T = emit_p2(b, g1T)
                h2 = emit_p3(b, a1T)
                a2T = emit_p4(b, h2, head=(b == 0))
                emit_p5(b, a2T, h3p, ip)

            def emit_pair(b0, loaded_x8, loaded_xp):
                # prefetch the next pair's inputs
                if b0 + 2 < bl and (b0 + 2) not in loaded_xp:
                    loaded_xp[b0 + 2] = emit_load_xpair(b0 + 2, 2)
                    if (b0 + 2) not in loaded_x8:
                        loaded_x8[b0 + 2] = emit_load_x8(b0 + 2)
                        loaded_x8[b0 + 3] = emit_load_x8(b0 + 3)
                h3p = hpp.tile([P, kt2, 2, 2 * c], F8, tag="hpair",
                               name=f"h3p_{b0}")
                # phase 1 of the NEXT pair's items is interleaved into this
                # pair so the PE never drains at pair boundaries
                emit_mid(b0, loaded_g.pop(b0), h3p, 0)
                if b0 + 2 < bl:
                    loaded_g[b0 + 2] = emit_p1(b0 + 2, loaded_x8[b0 + 2])
                emit_mid(b0 + 1, loaded_g.pop(b0 + 1), h3p, 1)
                if b0 + 3 < bl:
                    loaded_g[b0 + 3] = emit_p1(b0 + 3, loaded_x8[b0 + 3])
                emit_p6(b0, loaded_xp[b0], h3p, 2)

            loaded_x8 = {}
            loaded_xp = {}
            loaded_g = {}
            if paired:
                loaded_x8[0] = x8_first
                loaded_x8[1] = emit_load_x8(1)
                loaded_g[0] = emit_p1(0, loaded_x8[0], head=True)
                emit_load_at_tm()
                loaded_g[1] = emit_p1(1, loaded_x8[1])
                if bl >= 4:
                    loaded_x8[2] = emit_load_x8(2)
                    loaded_x8[3] = emit_load_x8(3)
                loaded_xp[0] = emit_load_xpair(0, 2)
                for b0 in range(0, bl, 2):
                    emit_pair(b0, loaded_x8, loaded_xp)
            else:
                emit_load_at_tm()
                for b in range(bl):
                    x8 = emit_load_x8(b)
                    xp = emit_load_xpair(b, 1)
                    h3p = hpp.tile([P, kt2, 2, c], F8, tag="hpair",
                                   name=f"h3p_{b}")
                    g1T = emit_p1(b, x8)
                    emit_mid(b, g1T, h3p, 0)
                    emit_p6(b, xp, h3p, 1)

    nc.compile()
    return nc


def _norm_adj_T(edges, n, npad, bias_row):
    """A^T padded to [npad, n] in fp32. AT[m, j] = A[j, m]; edge (r -> c)
    contributes dinv[r]*dinv[c] at AT[r, c]. Self loops included.
    If bias_row, AT[n, :n] = 1 (bias fold)."""
    row = np.concatenate([edges[0], np.arange(n, dtype=np.int64)])
    col = np.concatenate([edges[1], np.arange(n, dtype=np.int64)])
    deg = np.bincount(col, minlength=n).astype(np.float32)
    dinv = np.zeros(n, np.float32)
    nz = deg > 0
    dinv[nz] = 1.0 / np.sqrt(deg[nz])
    norm = dinv[row] * dinv[col]
    at = np.zeros((npad, n), np.float32)
    np.add.at(at, (row, col), norm)
    if bias_row:
        at[n, :n] = 1.0
    return at


def _dr_tiles(a, kt2):
    """[kt2*256, F] -> [P, kt2, 2, F] so [p, k2, i, :] = a[k2*256+i*128+p]."""
    return np.ascontiguousarray(
        a.reshape(kt2, 2, P, a.shape[-1]).transpose(2, 0, 1, 3))


_PROGRAM_CACHE = {}


def _get_program(bl, n, c):
    key = (bl, n, c)
    if key not in _PROGRAM_CACHE:
        _PROGRAM_CACHE[key] = build_program(bl, n, c)
    return _PROGRAM_CACHE[key]


def run(inputs, trace=False, n_cores=N_CORES):
    x = np.asarray(inputs["x"], dtype=np.float32)
    w1 = np.asarray(inputs["W1"], np.float32)
    w2 = np.asarray(inputs["W2"], np.float32)
    w3 = np.asarray(inputs["W3"], np.float32)
    b1 = np.asarray(inputs["b1"], np.float32)
    b2 = np.asarray(inputs["b2"], np.float32)
    b3 = np.asarray(inputs["b3"], np.float32)
    e_sp = np.asarray(inputs["keypoint_line_without_temporal"]).astype(np.int64)
    e_tm = np.asarray(inputs["keypoint_line_with_temporal"]).astype(np.int64)

    b_total, n, c = x.shape
    bl = b_total // n_cores
    kt = -(-(n + 1) // P)
    if kt % 2:
        kt += 1
    npad = kt * P
    kt2 = kt // 2
    paired = bl % 2 == 0

    nc = _get_program(bl, n, c)

    xpad = np.zeros((b_total, npad, c), np.float32)
    xpad[:, :n, :] = x
    x8 = np.stack([_dr_tiles(xpad[i].astype(NP_F8), kt2)
                   for i in range(b_total)])

    def _w_tiles(w):
        return np.ascontiguousarray(
            w.astype(NP_F8).reshape(2, P, c).transpose(1, 0, 2))

    nl = -(-n // 16) * 16

    def _at(at):
        atp = np.zeros((npad, nl), np.float32)
        atp[:, :n] = at
        return _dr_tiles(atp.astype(NP_F8), kt2)

    shared = {
        "at_sp": _at(_norm_adj_T(e_sp, n, npad, True)),
        "at_tm": _at(_norm_adj_T(e_tm, n, npad, False)),
        "w1": _w_tiles(w1),
        "w2": _w_tiles(w2),
        "w3": _w_tiles(w3),
        "b1": np.ascontiguousarray(b1.reshape(2, P).T),
        "b2": np.ascontiguousarray(b2.reshape(2, P).T),
        "b3": np.ascontiguousarray(b3.astype(NP_F8)[None, :]),
    }
    xbf = x.astype(NP_BF16)
    in_maps = [
        {"x8": np.ascontiguousarray(x8[i * bl:(i + 1) * bl]),
         "xbf": np.ascontiguousarray(xbf[i * bl:(i + 1) * bl]),
         **shared}
        for i in range(n_cores)
    ]
    res = run_bass_kernel_spmd(nc, in_maps, core_ids=list(range(n_cores)),
                               trace=trace)
    outs = []
    for r in res.results:
        o = r["out"].astype(np.float32)
        if paired:
            # [bl//2, n, 2, c] -> [bl, n, c]
            o = o.transpose(0, 2, 1, 3).reshape(bl, n, c)
        outs.append(o)
    out = np.concatenate(outs, axis=0)
    return out, res


def kernel(**inputs) -> np.ndarray:
    out, _ = run(inputs, trace=False)
    return out
